# revision 12
# baseline (speedup 1.0000x reference)
"""GCN residual block (2x GCNConv + relu, residual mean) on 8 Trainium2 cores.

Math (reference):
    A_hat = D^-1/2 (A + I) D^-1/2,  deg = indeg + 1
    h1 = relu((A_hat x) W1 + b1)        [uses A_hat @ (x W1) == (A_hat x) W1]
    h2 = relu((A_hat h1) W2 + b2)
    out = (x + h2) * 0.5

Device decomposition (per core c, nodes sharded by dst range):
    xs = dis * x (host),  z1 = dis^2 * x (host)
    seg1_i = sum_{j->i} xs_j               (gather + free-dim reduce)
    agg1 = dis*seg1 + z1                   (one DVE stt op)
    y1 = relu((agg1 W1 + b1)) * dis        (PE matmul + ACT relu w/ scale)
    AllGather y1 -> full table
    seg2_i = sum_{j->i} y1_j
    agg2 = dis*(seg2 + y1_i)
    h2 = relu(agg2 W2 + b2)
    host: out = 0.5*(x + h2)

HYBRID gather: the [128,1]-offset indirect DMA costs ~1.1us Pool desc-gen
per slot column but only ~0.30us/col of DMA-engine time; the bulk
dma_gather costs ~0.18us/col Pool but ~0.86us/col DMA (2-hop bounce) and
needs a lo/hi table split (int16 indices) costing ~1.5x slot padding.
Sending ~4/7 of batches down the indirect path and ~3/7 down dma_gather
loads Pool and the DMA engines evenly (~1.2ms/layer each, overlapped)
instead of bottlenecking one of them (4.7ms all-indirect, 5.5ms
all-gather).
"""
import sys

sys.path.insert(0, "/opt/trn_rl_repo")

import numpy as np

N = 50000
E = 1600000
F = 128
NCORES = 8
NSHARD = N // NCORES  # 6250
BATCHES = 49
SHARD = BATCHES * 128  # 6272 padded shard rows
TABROWS = NCORES * SHARD  # 50176
ZROW = 6256  # a guaranteed all-zero (dummy) row in core 0's section
HALF = 32768  # lo/hi table split for dma_gather (int16 index limit)
ZLO = 6256  # zero row in lo half
ZHI = 37610 - HALF  # zero row in hi half (core 5 tail), rebased
GW = 8  # max slot columns per dma_gather call (1024 idxs; ring cap ~72 descs)

LAST_RESULTS = None  # BassKernelResults of the most recent run (for test.py)


def _is_gather_batch(b):
    # ~3/7 of batches via dma_gather, interleaved so Pool & DMA stay co-busy
    return (b % 7) >= 4


def _split_calls(width):
    if width == 0:
        return []
    ncalls = -(-width // GW)
    base = width // ncalls
    rem = width - base * ncalls
    return [base + 1] * rem + [base] * (ncalls - rem)


def _preprocess(x, edges):
    """Host-side graph prep. Returns per-core tensors + shared batch plan."""
    src = edges[0].astype(np.int64)
    dst = edges[1].astype(np.int64)

    deg = np.bincount(dst, minlength=N).astype(np.float32) + 1.0
    dis = (1.0 / np.sqrt(deg)).astype(np.float32)

    perm_rows = np.empty(N, dtype=np.int64)  # node -> table row
    order_per_core = []
    for c in range(NCORES):
        nodes = np.arange(c * NSHARD, (c + 1) * NSHARD, dtype=np.int64)
        order = nodes[np.argsort(-deg[nodes], kind="stable")]
        order_per_core.append(order)
        perm_rows[order] = c * SHARD + np.arange(NSHARD)

    xs_tab = np.zeros((TABROWS, F), dtype=np.float32)
    z1_tabs = []
    dis_tiles = []
    for c in range(NCORES):
        order = order_per_core[c]
        rows = c * SHARD + np.arange(NSHARD)
        xs_tab[rows] = dis[order, None] * x[order]
        z1 = np.zeros((SHARD, F), dtype=np.float32)
        z1[:NSHARD] = dis[order, None] ** 2 * x[order]
        z1_tabs.append(z1)
        dt = np.zeros(SHARD, dtype=np.float32)
        dt[:NSHARD] = dis[order]
        dis_tiles.append(dt.reshape(BATCHES, 128).T.copy())  # [128, BATCHES]

    psrc = perm_rows[src]
    pdst = perm_rows[dst]
    o = np.argsort(pdst, kind="stable")
    psrc_s = psrc[o]
    pdst_s = pdst[o]
    counts = np.bincount(pdst_s, minlength=TABROWS)
    indptr = np.concatenate([[0], np.cumsum(counts)])

    # padded per-(core,batch) source matrices + per-batch capacities
    V_all, M_all, T_all = [], [], []
    d_hi = np.zeros(BATCHES, dtype=np.int64)  # single-layout max (indirect)
    d_lo2 = np.zeros(BATCHES, dtype=np.int64)  # lo max (gather)
    d_hi2 = np.zeros(BATCHES, dtype=np.int64)  # hi max (gather)
    for c in range(NCORES):
        Vc, Mc, Tc = [], [], []
        for b in range(BATCHES):
            rows = c * SHARD + b * 128 + np.arange(128)
            cnt = counts[rows]
            starts = indptr[rows]
            d = max(int(cnt.max()), 1)
            s_grid = np.arange(d)[None, :]
            take = s_grid < cnt[:, None]
            gpos = starts[:, None] + np.minimum(
                s_grid, np.maximum(cnt[:, None] - 1, 0)
            )
            gpos = np.minimum(gpos, max(len(psrc_s) - 1, 0))
            vals = psrc_s[gpos]
            lo_m = take & (vals < HALF)
            Vc.append(vals)
            Mc.append(lo_m)
            Tc.append(take)
            d_hi[b] = max(d_hi[b], d)
            d_lo2[b] = max(d_lo2[b], int(lo_m.sum(axis=1).max()))
            d_hi2[b] = max(d_hi2[b], int((take & ~lo_m).sum(axis=1).max()))
        V_all.append(Vc)
        M_all.append(Mc)
        T_all.append(Tc)
    d_lo2 = np.maximum(d_lo2, 1)
    d_hi2 = np.maximum(d_hi2, 1)

    # batch plan, shared across cores
    plan = []  # per batch: dict(kind, ...)
    icol = 0  # running column in the i32 indirect idx tensor
    wcol = 0  # running column in the wrapped i16 gather idx tensor
    for b in range(BATCHES):
        if _is_gather_batch(b):
            dlo, dhi = int(d_lo2[b]), int(d_hi2[b])
            calls = []
            col = 0
            for w in _split_calls(dlo):
                calls.append((True, col, w, wcol))
                col += w
                wcol += 8 * w
            col = dlo
            for w in _split_calls(dhi):
                calls.append((False, col, w, wcol))
                col += w
                wcol += 8 * w
            plan.append(dict(kind="g", b=b, dlo=dlo, dhi=dhi, calls=calls))
        else:
            d = int(d_hi[b])
            plan.append(dict(kind="i", b=b, d=d, off=icol))
            icol += d
    icols, wcols = icol, wcol
    maxtile = max(
        max((p["d"] for p in plan if p["kind"] == "i"), default=1),
        max((p["dlo"] + p["dhi"] for p in plan if p["kind"] == "g"), default=1),
    )

    # per-core index tensors
    idx_tiles = []  # [128, icols] int32 (indirect batches)
    idxw_tiles = []  # [128, wcols] int16 wrapped (gather batches)
    for c in range(NCORES):
        idx = np.full((128, max(icols, 1)), ZROW, dtype=np.int32)
        wrapped = np.empty((16, max(wcols, 1)), dtype=np.int16)
        for p in plan:
            b = p["b"]
            vals, lo_m, take = V_all[c][b], M_all[c][b], T_all[c][b]
            if p["kind"] == "i":
                d, off = p["d"], p["off"]
                dv = vals.shape[1]
                padded = np.where(take, vals, ZROW)
                if dv < d:
                    padded = np.concatenate(
                        [padded, np.full((128, d - dv), ZROW, dtype=np.int64)],
                        axis=1,
                    )
                idx[:, off : off + d] = padded[:, :d]
            else:
                dlo, dhi = p["dlo"], p["dhi"]
                hi_m = take & ~lo_m
                ordl = np.argsort(~lo_m, axis=1, kind="stable")
                lov = np.take_along_axis(vals, ordl, axis=1)
                locnt = lo_m.sum(axis=1)
                dl = lov.shape[1]
                lo_pad = np.where(np.arange(dl)[None, :] < locnt[:, None], lov, ZLO)
                if dl < dlo:
                    lo_pad = np.concatenate(
                        [lo_pad, np.full((128, dlo - dl), ZLO, dtype=np.int64)],
                        axis=1,
                    )
                ordh = np.argsort(~hi_m, axis=1, kind="stable")
                hiv = np.take_along_axis(vals, ordh, axis=1) - HALF
                hicnt = hi_m.sum(axis=1)
                dh = hiv.shape[1]
                hi_pad = np.where(np.arange(dh)[None, :] < hicnt[:, None], hiv, ZHI)
                if dh < dhi:
                    hi_pad = np.concatenate(
                        [hi_pad, np.full((128, dhi - dh), ZHI, dtype=np.int64)],
                        axis=1,
                    )
                slot_rows = np.concatenate(
                    [lo_pad[:, :dlo], hi_pad[:, :dhi]], axis=1
                )  # [128, dlo+dhi]
                for (is_lo, col0, w, wc0) in p["calls"]:
                    L = slot_rows[:, col0 : col0 + w].T.reshape(-1)
                    blk = L.reshape(8 * w, 16).T.astype(np.int16)
                    wrapped[:, wc0 : wc0 + 8 * w] = blk
        idx_tiles.append(idx)
        idxw_tiles.append(np.tile(wrapped, (8, 1)))

    return (
        xs_tab,
        z1_tabs,
        dis_tiles,
        idx_tiles,
        idxw_tiles,
        plan,
        icols,
        wcols,
        maxtile,
        order_per_core,
    )


def _build(plan, icols, wcols, maxtile):
    from concourse import bacc, bass, mybir, tile
    from concourse.masks import make_identity

    f32 = mybir.dt.float32
    i32 = mybir.dt.int32
    i16 = mybir.dt.int16

    nc = bacc.Bacc("TRN2", target_bir_lowering=False, debug=False, num_devices=NCORES)

    xs = nc.dram_tensor("xs", [TABROWS, F], f32, kind="ExternalInput")
    z1 = nc.dram_tensor("z1", [SHARD, F], f32, kind="ExternalInput")
    idx = nc.dram_tensor("idx", [128, max(icols, 1)], i32, kind="ExternalInput")
    idxw = nc.dram_tensor("idxw", [128, max(wcols, 1)], i16, kind="ExternalInput")
    dis = nc.dram_tensor("dis", [128, BATCHES], f32, kind="ExternalInput")
    w1 = nc.dram_tensor("w1", [F, F], f32, kind="ExternalInput")
    b1 = nc.dram_tensor("b1", [1, F], f32, kind="ExternalInput")
    w2 = nc.dram_tensor("w2", [F, F], f32, kind="ExternalInput")
    b2 = nc.dram_tensor("b2", [1, F], f32, kind="ExternalInput")
    h2 = nc.dram_tensor("h2", [SHARD, F], f32, kind="ExternalOutput")

    y1_local = nc.dram_tensor("y1_local", [SHARD, F], f32)
    y1_full = nc.dram_tensor("y1_full", [TABROWS, F], f32, addr_space="Shared")

    with tile.TileContext(nc) as tc:
        with (
            tc.tile_pool(name="const", bufs=1) as cpool,
            tc.tile_pool(name="y1pool", bufs=BATCHES) as ypool,
            tc.tile_pool(name="work", bufs=3) as pool,
            tc.tile_pool(name="slots", bufs=2) as spool,
            tc.tile_pool(name="psum", bufs=2, space="PSUM") as psum,
        ):
            ident = cpool.tile([128, 128], f32)
            make_identity(nc, ident[:])
            ones = cpool.tile([1, 128], f32)
            nc.gpsimd.memset(ones[:], 1.0)

            idx_s = cpool.tile([128, max(icols, 1)], i32)
            nc.sync.dma_start(out=idx_s[:], in_=idx[:])
            idxw_s = cpool.tile([128, max(wcols, 1)], i16)
            nc.sync.dma_start(out=idxw_s[:], in_=idxw[:])
            dis_s = cpool.tile([128, BATCHES], f32)
            nc.sync.dma_start(out=dis_s[:], in_=dis[:])
            w1_s = cpool.tile([F, F], f32)
            nc.sync.dma_start(out=w1_s[:], in_=w1[:])
            b1_s = cpool.tile([1, F], f32)
            nc.sync.dma_start(out=b1_s[:], in_=b1[:])
            w2_s = cpool.tile([F, F], f32)
            nc.sync.dma_start(out=w2_s[:], in_=w2[:])
            b2_s = cpool.tile([1, F], f32)
            nc.sync.dma_start(out=b2_s[:], in_=b2[:])

            y1_tiles = []

            def layer(table_ap, wt, bt, self_src, out_sink, first):
                tab_lo = table_ap[:HALF, :]
                tab_hi = table_ap[HALF:, :]
                for p in plan:
                    b = p["b"]
                    slots = spool.tile([128, maxtile, F], f32, tag="slots")
                    if p["kind"] == "i":
                        d, off = p["d"], p["off"]
                        for s in range(d):
                            col = off + s
                            nc.gpsimd.indirect_dma_start(
                                out=slots[:, s, :],
                                out_offset=None,
                                in_=table_ap,
                                in_offset=bass.IndirectOffsetOnAxis(
                                    ap=idx_s[:, col : col + 1], axis=0
                                ),
                            )
                        seg = pool.tile([128, F], f32, tag="seg")
                        nc.vector.tensor_reduce(
                            out=seg[:],
                            in_=slots[:, :d, :].rearrange("p d f -> p f d"),
                            axis=mybir.AxisListType.X,
                            op=mybir.AluOpType.add,
                        )
                    else:
                        dlo, dhi = p["dlo"], p["dhi"]
                        for (is_lo, col0, w, wc0) in p["calls"]:
                            nc.gpsimd.dma_gather(
                                out_ap=slots[:, col0 : col0 + w, :],
                                in_ap=tab_lo if is_lo else tab_hi,
                                idxs_ap=idxw_s[:, wc0 : wc0 + 8 * w],
                                num_idxs=128 * w,
                                num_idxs_reg=128 * w,
                                elem_size=F,
                            )
                        segl = pool.tile([128, F], f32, tag="segl")
                        nc.vector.tensor_reduce(
                            out=segl[:],
                            in_=slots[:, :dlo, :].rearrange("p d f -> p f d"),
                            axis=mybir.AxisListType.X,
                            op=mybir.AluOpType.add,
                        )
                        segh = pool.tile([128, F], f32, tag="segh")
                        nc.vector.tensor_reduce(
                            out=segh[:],
                            in_=slots[:, dlo : dlo + dhi, :].rearrange(
                                "p d f -> p f d"
                            ),
                            axis=mybir.AxisListType.X,
                            op=mybir.AluOpType.add,
                        )
                        seg = pool.tile([128, F], f32, tag="seg")
                        nc.vector.tensor_tensor(
                            out=seg[:],
                            in0=segl[:],
                            in1=segh[:],
                            op=mybir.AluOpType.add,
                        )
                    agg = pool.tile([128, F], f32, tag="agg")
                    if first:
                        zt = pool.tile([128, F], f32, tag="zt")
                        nc.sync.dma_start(
                            out=zt[:], in_=z1[b * 128 : (b + 1) * 128, :]
                        )
                        nc.vector.scalar_tensor_tensor(
                            out=agg[:],
                            in0=seg[:],
                            scalar=dis_s[:, b : b + 1],
                            in1=zt[:],
                            op0=mybir.AluOpType.mult,
                            op1=mybir.AluOpType.add,
                        )
                    else:
                        t = pool.tile([128, F], f32, tag="t2")
                        nc.vector.tensor_tensor(
                            out=t[:],
                            in0=seg[:],
                            in1=self_src[b][:],
                            op=mybir.AluOpType.add,
                        )
                        nc.vector.tensor_scalar_mul(
                            out=agg[:], in0=t[:], scalar1=dis_s[:, b : b + 1]
                        )
                    psumT = psum.tile([128, 128], f32, tag="pt")
                    nc.tensor.transpose(out=psumT[:], in_=agg[:], identity=ident[:])
                    aggT = pool.tile([128, 128], f32, tag="aggT")
                    nc.scalar.activation(
                        out=aggT[:],
                        in_=psumT[:],
                        func=mybir.ActivationFunctionType.Copy,
                    )
                    ph = psum.tile([128, F], f32, tag="ph")
                    nc.tensor.matmul(
                        ph[:], lhsT=ones[:], rhs=bt[:], start=True, stop=False
                    )
                    nc.tensor.matmul(
                        ph[:], lhsT=aggT[:], rhs=wt[:], start=False, stop=True
                    )
                    if first:
                        y1t = ypool.tile([128, F], f32, tag="y1")
                        nc.scalar.activation(
                            out=y1t[:],
                            in_=ph[:],
                            func=mybir.ActivationFunctionType.Relu,
                            scale=dis_s[:, b : b + 1],
                        )
                        y1_tiles.append(y1t)
                        nc.sync.dma_start(
                            out=y1_local[b * 128 : (b + 1) * 128, :], in_=y1t[:]
                        )
                    else:
                        h2t = pool.tile([128, F], f32, tag="h2t")
                        nc.scalar.activation(
                            out=h2t[:],
                            in_=ph[:],
                            func=mybir.ActivationFunctionType.Relu,
                        )
                        nc.sync.dma_start(
                            out=out_sink[b * 128 : (b + 1) * 128, :], in_=h2t[:]
                        )

            layer(xs[:], w1_s, b1_s, None, None, first=True)

            nc.gpsimd.collective_compute(
                "AllGather",
                bass.mybir.AluOpType.bypass,
                replica_groups=[list(range(NCORES))],
                ins=[y1_local[:]],
                outs=[y1_full[:]],
            )

            layer(y1_full[:], w2_s, b2_s, y1_tiles, h2, first=False)

    nc.compile()
    return nc


def _ensure_ntff_hook():
    """Register the axon NTFF profiling hook if the environment's antenv
    stub lacks the axon_hooks module (otherwise trace=True raises)."""
    import sys as _sys
    import types as _types

    try:
        from antenv.axon_hooks import (  # noqa: F401
            get_axon_ntff_profile_hook,
            set_axon_ntff_profile_hook,
        )
    except ImportError:
        mod = _types.ModuleType("antenv.axon_hooks")
        mod._hook = None

        def set_axon_ntff_profile_hook(h, _mod=mod):
            _mod._hook = h

        def get_axon_ntff_profile_hook(_mod=mod):
            return _mod._hook

        mod.set_axon_ntff_profile_hook = set_axon_ntff_profile_hook
        mod.get_axon_ntff_profile_hook = get_axon_ntff_profile_hook
        _sys.modules["antenv.axon_hooks"] = mod
        try:
            import antenv

            antenv.axon_hooks = mod
        except ImportError:
            pass
    else:
        if get_axon_ntff_profile_hook() is not None:
            return
        from antenv.axon_hooks import set_axon_ntff_profile_hook
    try:
        from trn_agent_boot.trn_boot import _ntff_profile_via_ctypes

        hook = _ntff_profile_via_ctypes("/opt/axon/libaxon_pjrt.so")
        if hook is not None:
            set_axon_ntff_profile_hook(hook)
    except Exception:
        pass  # tracing degrades; compile + run still work


def kernel(x, edges, W1, b1, W2, b2):
    global LAST_RESULTS
    import os

    from concourse.bass_utils import run_bass_kernel_spmd

    _ensure_ntff_hook()

    x = np.asarray(x, dtype=np.float32)
    edges = np.asarray(edges)
    (
        xs_tab,
        z1_tabs,
        dis_tiles,
        idx_tiles,
        idxw_tiles,
        plan,
        icols,
        wcols,
        maxtile,
        order_per_core,
    ) = _preprocess(x, edges)

    nc = _build(plan, icols, wcols, maxtile)

    w1 = np.asarray(W1, dtype=np.float32)
    w2 = np.asarray(W2, dtype=np.float32)
    b1v = np.asarray(b1, dtype=np.float32).reshape(1, F)
    b2v = np.asarray(b2, dtype=np.float32).reshape(1, F)

    in_maps = []
    for c in range(NCORES):
        in_maps.append(
            {
                "xs": xs_tab,
                "z1": z1_tabs[c],
                "idx": idx_tiles[c],
                "idxw": idxw_tiles[c],
                "dis": dis_tiles[c],
                "w1": w1,
                "b1": b1v,
                "w2": w2,
                "b2": b2v,
            }
        )

    trace = os.environ.get("BASS_TRACE", "1") == "1"
    res = run_bass_kernel_spmd(nc, in_maps, list(range(NCORES)), trace=trace)
    LAST_RESULTS = res

    h2_full = np.empty((N, F), dtype=np.float32)
    for c in range(NCORES):
        h2c = res.results[c]["h2"][:NSHARD]
        h2_full[order_per_core[c]] = h2c
    return (0.5 * (x + h2_full)).astype(np.float32)


# revision 13
# speedup vs baseline: 1.0970x; 1.0970x over previous
"""GCN residual block (2x GCNConv + relu, residual mean) on 8 Trainium2 cores.

Math (reference):
    A_hat = D^-1/2 (A + I) D^-1/2,  deg = indeg + 1
    h1 = relu((A_hat x) W1 + b1)        [uses A_hat @ (x W1) == (A_hat x) W1]
    h2 = relu((A_hat h1) W2 + b2)
    out = (x + h2) * 0.5

Device decomposition (per core c, nodes sharded by dst range):
    xs = dis * x (host),  z1 = dis^2 * x (host)
    seg1_i = sum_{j->i} xs_j               (gather + free-dim reduce)
    agg1 = dis*seg1 + z1                   (one DVE stt op)
    y1 = relu((agg1 W1 + b1)) * dis        (PE matmul + ACT relu w/ scale)
    AllGather y1 -> full table
    seg2_i = sum_{j->i} y1_j
    agg2 = dis*(seg2 + y1_i)
    h2 = relu(agg2 W2 + b2)
    host: out = 0.5*(x + h2)

Nodes are permuted per-core by degree (descending) so 128-node batches have
near-uniform slot counts; gathers use the production [128,1]-offset
indirect DMA (one column of slots per call).  Measured on HW: the Pool
engine's SWDGE desc-gen (~1.1us/call) is the bottleneck; the bulk
dma_gather alternative moves bytes through a 2-hop bounce and measured
slower end-to-end, and multi-column indirect offsets mis-lower in ucode.
"""
import sys

sys.path.insert(0, "/opt/trn_rl_repo")

import numpy as np

N = 50000
E = 1600000
F = 128
NCORES = 8
NSHARD = N // NCORES  # 6250
BATCHES = 49
SHARD = BATCHES * 128  # 6272 padded shard rows
TABROWS = NCORES * SHARD  # 50176
ZROW = 6256  # a guaranteed all-zero (dummy) row in core 0's section

LAST_RESULTS = None  # BassKernelResults of the most recent run (for test.py)


def _preprocess(x, edges):
    """Host-side graph prep. Returns per-core index/scale tensors + tables."""
    src = edges[0].astype(np.int64)
    dst = edges[1].astype(np.int64)

    deg = np.bincount(dst, minlength=N).astype(np.float32) + 1.0
    dis = (1.0 / np.sqrt(deg)).astype(np.float32)

    # permute: within each core's shard, sort nodes by in-degree descending
    perm_rows = np.empty(N, dtype=np.int64)  # node -> table row
    order_per_core = []
    for c in range(NCORES):
        nodes = np.arange(c * NSHARD, (c + 1) * NSHARD, dtype=np.int64)
        order = nodes[np.argsort(-deg[nodes], kind="stable")]
        order_per_core.append(order)
        perm_rows[order] = c * SHARD + np.arange(NSHARD)

    # tables in permuted order (zero rows at each core's tail)
    xs_tab = np.zeros((TABROWS, F), dtype=np.float32)
    z1_tabs = []
    dis_tiles = []
    for c in range(NCORES):
        order = order_per_core[c]
        rows = c * SHARD + np.arange(NSHARD)
        xs_tab[rows] = dis[order, None] * x[order]
        z1 = np.zeros((SHARD, F), dtype=np.float32)
        z1[:NSHARD] = dis[order, None] ** 2 * x[order]
        z1_tabs.append(z1)
        dt = np.zeros(SHARD, dtype=np.float32)
        dt[:NSHARD] = dis[order]
        dis_tiles.append(dt.reshape(BATCHES, 128).T.copy())  # [128, BATCHES]

    # per-core CSR of in-edges in permuted node order
    psrc = perm_rows[src]  # source table rows
    pdst = perm_rows[dst]
    o = np.argsort(pdst, kind="stable")
    psrc_s = psrc[o]
    pdst_s = pdst[o]
    counts = np.bincount(pdst_s, minlength=TABROWS)
    indptr = np.concatenate([[0], np.cumsum(counts)])

    # batch slot capacities, shared across cores: d_hi[b] = max over cores
    cpb = counts.reshape(NCORES, BATCHES, 128)
    d_hi = cpb.max(axis=(0, 2)).astype(np.int64)  # [BATCHES]
    sumd = int(d_hi.sum())

    idx_tiles = []
    for c in range(NCORES):
        idx = np.full((128, sumd), ZROW, dtype=np.int32)
        off = 0
        for b in range(BATCHES):
            rows = c * SHARD + b * 128 + np.arange(128)
            d = d_hi[b]
            # fill idx[p, off+s] = s-th in-edge source of node rows[p]
            cnt = counts[rows]
            starts = indptr[rows]
            s_grid = np.arange(d)[None, :]
            take = s_grid < cnt[:, None]
            gather_pos = starts[:, None] + np.minimum(s_grid, np.maximum(cnt[:, None] - 1, 0))
            gather_pos = np.minimum(gather_pos, max(len(psrc_s) - 1, 0))
            vals = psrc_s[gather_pos] if len(psrc_s) else np.zeros_like(gather_pos)
            idx[:, off : off + d] = np.where(take, vals, ZROW)
            off += d
        idx_tiles.append(idx)

    return xs_tab, z1_tabs, dis_tiles, idx_tiles, d_hi, order_per_core


def _build(d_hi):
    from concourse import bacc, bass, mybir, tile
    from concourse.masks import make_identity

    f32 = mybir.dt.float32
    i32 = mybir.dt.int32
    sumd = int(d_hi.sum())

    nc = bacc.Bacc("TRN2", target_bir_lowering=False, debug=False, num_devices=NCORES)

    xs = nc.dram_tensor("xs", [TABROWS, F], f32, kind="ExternalInput")
    z1 = nc.dram_tensor("z1", [SHARD, F], f32, kind="ExternalInput")
    idx = nc.dram_tensor("idx", [128, sumd], i32, kind="ExternalInput")
    dis = nc.dram_tensor("dis", [128, BATCHES], f32, kind="ExternalInput")
    w1 = nc.dram_tensor("w1", [F, F], f32, kind="ExternalInput")
    b1 = nc.dram_tensor("b1", [1, F], f32, kind="ExternalInput")
    w2 = nc.dram_tensor("w2", [F, F], f32, kind="ExternalInput")
    b2 = nc.dram_tensor("b2", [1, F], f32, kind="ExternalInput")
    h2 = nc.dram_tensor("h2", [SHARD, F], f32, kind="ExternalOutput")

    y1_local = nc.dram_tensor("y1_local", [SHARD, F], f32)
    y1_full = nc.dram_tensor("y1_full", [TABROWS, F], f32, addr_space="Shared")

    with tile.TileContext(nc) as tc:
        with (
            tc.tile_pool(name="const", bufs=1) as cpool,
            tc.tile_pool(name="y1pool", bufs=BATCHES) as ypool,
            tc.tile_pool(name="work", bufs=3) as pool,
            tc.tile_pool(name="slots", bufs=2) as spool,
            tc.tile_pool(name="psum", bufs=2, space="PSUM") as psum,
        ):
            ident = cpool.tile([128, 128], f32)
            make_identity(nc, ident[:])
            ones = cpool.tile([1, 128], f32)
            nc.gpsimd.memset(ones[:], 1.0)

            idx_s = cpool.tile([128, sumd], i32)
            nc.sync.dma_start(out=idx_s[:], in_=idx[:])
            dis_s = cpool.tile([128, BATCHES], f32)
            nc.sync.dma_start(out=dis_s[:], in_=dis[:])
            w1_s = cpool.tile([F, F], f32)
            nc.sync.dma_start(out=w1_s[:], in_=w1[:])
            b1_s = cpool.tile([1, F], f32)
            nc.sync.dma_start(out=b1_s[:], in_=b1[:])
            w2_s = cpool.tile([F, F], f32)
            nc.sync.dma_start(out=w2_s[:], in_=w2[:])
            b2_s = cpool.tile([1, F], f32)
            nc.sync.dma_start(out=b2_s[:], in_=b2[:])

            offs = np.concatenate([[0], np.cumsum(d_hi)]).astype(int)
            y1_tiles = []

            def layer(table_ap, wt, bt, self_src, out_sink, first):
                for b in range(BATCHES):
                    d = int(d_hi[b])
                    slots = spool.tile([128, d, F], f32, tag="slots")
                    for s in range(d):
                        col = int(offs[b]) + s
                        nc.gpsimd.indirect_dma_start(
                            out=slots[:, s, :],
                            out_offset=None,
                            in_=table_ap,
                            in_offset=bass.IndirectOffsetOnAxis(
                                ap=idx_s[:, col : col + 1], axis=0
                            ),
                        )
                    seg = pool.tile([128, F], f32, tag="seg")
                    nc.vector.tensor_reduce(
                        out=seg[:],
                        in_=slots[:].rearrange("p d f -> p f d"),
                        axis=mybir.AxisListType.X,
                        op=mybir.AluOpType.add,
                    )
                    agg = pool.tile([128, F], f32, tag="agg")
                    if first:
                        zt = pool.tile([128, F], f32, tag="zt")
                        nc.sync.dma_start(
                            out=zt[:], in_=z1[b * 128 : (b + 1) * 128, :]
                        )
                        nc.vector.scalar_tensor_tensor(
                            out=agg[:],
                            in0=seg[:],
                            scalar=dis_s[:, b : b + 1],
                            in1=zt[:],
                            op0=mybir.AluOpType.mult,
                            op1=mybir.AluOpType.add,
                        )
                    else:
                        t = pool.tile([128, F], f32, tag="t2")
                        nc.vector.tensor_tensor(
                            out=t[:],
                            in0=seg[:],
                            in1=self_src[b][:],
                            op=mybir.AluOpType.add,
                        )
                        nc.vector.tensor_scalar_mul(
                            out=agg[:], in0=t[:], scalar1=dis_s[:, b : b + 1]
                        )
                    psumT = psum.tile([128, 128], f32, tag="pt")
                    nc.tensor.transpose(out=psumT[:], in_=agg[:], identity=ident[:])
                    aggT = pool.tile([128, 128], f32, tag="aggT")
                    nc.scalar.activation(
                        out=aggT[:],
                        in_=psumT[:],
                        func=mybir.ActivationFunctionType.Copy,
                    )
                    ph = psum.tile([128, F], f32, tag="ph")
                    nc.tensor.matmul(
                        ph[:], lhsT=ones[:], rhs=bt[:], start=True, stop=False
                    )
                    nc.tensor.matmul(
                        ph[:], lhsT=aggT[:], rhs=wt[:], start=False, stop=True
                    )
                    if first:
                        y1t = ypool.tile([128, F], f32, tag="y1")
                        nc.scalar.activation(
                            out=y1t[:],
                            in_=ph[:],
                            func=mybir.ActivationFunctionType.Relu,
                            scale=dis_s[:, b : b + 1],
                        )
                        y1_tiles.append(y1t)
                        nc.sync.dma_start(
                            out=y1_local[b * 128 : (b + 1) * 128, :], in_=y1t[:]
                        )
                    else:
                        h2t = pool.tile([128, F], f32, tag="h2t")
                        nc.scalar.activation(
                            out=h2t[:],
                            in_=ph[:],
                            func=mybir.ActivationFunctionType.Relu,
                        )
                        nc.sync.dma_start(
                            out=out_sink[b * 128 : (b + 1) * 128, :], in_=h2t[:]
                        )

            layer(xs[:], w1_s, b1_s, None, None, first=True)

            nc.gpsimd.collective_compute(
                "AllGather",
                bass.mybir.AluOpType.bypass,
                replica_groups=[list(range(NCORES))],
                ins=[y1_local[:]],
                outs=[y1_full[:]],
            )

            layer(y1_full[:], w2_s, b2_s, y1_tiles, h2, first=False)

    nc.compile()
    return nc


def _ensure_ntff_hook():
    """Register the axon NTFF profiling hook if the environment's antenv
    stub lacks the axon_hooks module (otherwise trace=True raises)."""
    import sys as _sys
    import types as _types

    try:
        from antenv.axon_hooks import (  # noqa: F401
            get_axon_ntff_profile_hook,
            set_axon_ntff_profile_hook,
        )
    except ImportError:
        mod = _types.ModuleType("antenv.axon_hooks")
        mod._hook = None

        def set_axon_ntff_profile_hook(h, _mod=mod):
            _mod._hook = h

        def get_axon_ntff_profile_hook(_mod=mod):
            return _mod._hook

        mod.set_axon_ntff_profile_hook = set_axon_ntff_profile_hook
        mod.get_axon_ntff_profile_hook = get_axon_ntff_profile_hook
        _sys.modules["antenv.axon_hooks"] = mod
        try:
            import antenv

            antenv.axon_hooks = mod
        except ImportError:
            pass
    else:
        if get_axon_ntff_profile_hook() is not None:
            return
        from antenv.axon_hooks import set_axon_ntff_profile_hook
    try:
        from trn_agent_boot.trn_boot import _ntff_profile_via_ctypes

        hook = _ntff_profile_via_ctypes("/opt/axon/libaxon_pjrt.so")
        if hook is not None:
            set_axon_ntff_profile_hook(hook)
    except Exception:
        pass  # tracing degrades; compile + run still work


def kernel(x, edges, W1, b1, W2, b2):
    global LAST_RESULTS
    import os

    from concourse.bass_utils import run_bass_kernel_spmd

    _ensure_ntff_hook()

    x = np.asarray(x, dtype=np.float32)
    edges = np.asarray(edges)
    xs_tab, z1_tabs, dis_tiles, idx_tiles, d_hi, order_per_core = _preprocess(x, edges)

    nc = _build(d_hi)

    w1 = np.asarray(W1, dtype=np.float32)
    w2 = np.asarray(W2, dtype=np.float32)
    b1v = np.asarray(b1, dtype=np.float32).reshape(1, F)
    b2v = np.asarray(b2, dtype=np.float32).reshape(1, F)

    in_maps = []
    for c in range(NCORES):
        in_maps.append(
            {
                "xs": xs_tab,
                "z1": z1_tabs[c],
                "idx": idx_tiles[c],
                "dis": dis_tiles[c],
                "w1": w1,
                "b1": b1v,
                "w2": w2,
                "b2": b2v,
            }
        )

    trace = os.environ.get("BASS_TRACE", "1") == "1"
    res = run_bass_kernel_spmd(nc, in_maps, list(range(NCORES)), trace=trace)
    LAST_RESULTS = res

    h2_full = np.empty((N, F), dtype=np.float32)
    for c in range(NCORES):
        h2c = res.results[c]["h2"][:NSHARD]
        h2_full[order_per_core[c]] = h2c
    return (0.5 * (x + h2_full)).astype(np.float32)


# revision 14
# speedup vs baseline: 1.0977x; 1.0007x over previous
"""GCN residual block (2x GCNConv + relu, residual mean) on 8 Trainium2 cores.

Math (reference):
    A_hat = D^-1/2 (A + I) D^-1/2,  deg = indeg + 1
    h1 = relu((A_hat x) W1 + b1)        [uses A_hat @ (x W1) == (A_hat x) W1]
    h2 = relu((A_hat h1) W2 + b2)
    out = (x + h2) * 0.5

Device decomposition (per core c, nodes sharded by dst range):
    xs = dis * x (host),  z1 = dis^2 * x (host)
    seg1_i = sum_{j->i} xs_j               (gather + free-dim reduce)
    agg1 = dis*seg1 + z1                   (one DVE stt op)
    y1 = relu((agg1 W1 + b1)) * dis        (PE matmul + ACT relu w/ scale)
    AllGather y1 -> full table
    seg2_i = sum_{j->i} y1_j
    agg2 = dis*(seg2 + y1_i)
    h2 = relu(agg2 W2 + b2)
    host: out = 0.5*(x + h2)

Nodes are permuted per-core by degree (descending) so 128-node batches have
near-uniform slot counts; gathers use the production [128,1]-offset
indirect DMA (one column of slots per call).  Measured on HW: the Pool
engine's SWDGE desc-gen (~1.1us/call) is the bottleneck; the bulk
dma_gather alternative moves bytes through a 2-hop bounce and measured
slower end-to-end, and multi-column indirect offsets mis-lower in ucode.
"""
import sys

sys.path.insert(0, "/opt/trn_rl_repo")

import numpy as np

N = 50000
E = 1600000
F = 128
NCORES = 8
NSHARD = N // NCORES  # 6250
BATCHES = 49
SHARD = BATCHES * 128  # 6272 padded shard rows
TABROWS = NCORES * SHARD  # 50176
ZROW = 6256  # a guaranteed all-zero (dummy) row in core 0's section

LAST_RESULTS = None  # BassKernelResults of the most recent run (for test.py)


def _preprocess(x, edges):
    """Host-side graph prep. Returns per-core index/scale tensors + tables."""
    src = edges[0].astype(np.int64)
    dst = edges[1].astype(np.int64)

    deg = np.bincount(dst, minlength=N).astype(np.float32) + 1.0
    dis = (1.0 / np.sqrt(deg)).astype(np.float32)

    # permute: within each core's shard, sort nodes by in-degree descending
    perm_rows = np.empty(N, dtype=np.int64)  # node -> table row
    order_per_core = []
    for c in range(NCORES):
        nodes = np.arange(c * NSHARD, (c + 1) * NSHARD, dtype=np.int64)
        order = nodes[np.argsort(-deg[nodes], kind="stable")]
        order_per_core.append(order)
        perm_rows[order] = c * SHARD + np.arange(NSHARD)

    # tables in permuted order (zero rows at each core's tail)
    xs_tab = np.zeros((TABROWS, F), dtype=np.float32)
    z1_tabs = []
    dis_tiles = []
    for c in range(NCORES):
        order = order_per_core[c]
        rows = c * SHARD + np.arange(NSHARD)
        xs_tab[rows] = dis[order, None] * x[order]
        z1 = np.zeros((SHARD, F), dtype=np.float32)
        z1[:NSHARD] = dis[order, None] ** 2 * x[order]
        z1_tabs.append(z1)
        dt = np.zeros(SHARD, dtype=np.float32)
        dt[:NSHARD] = dis[order]
        dis_tiles.append(dt.reshape(BATCHES, 128).T.copy())  # [128, BATCHES]

    # per-core CSR of in-edges in permuted node order
    psrc = perm_rows[src]  # source table rows
    pdst = perm_rows[dst]
    o = np.argsort(pdst, kind="stable")
    psrc_s = psrc[o]
    pdst_s = pdst[o]
    counts = np.bincount(pdst_s, minlength=TABROWS)
    indptr = np.concatenate([[0], np.cumsum(counts)])

    # batch slot capacities, shared across cores: d_hi[b] = max over cores
    cpb = counts.reshape(NCORES, BATCHES, 128)
    d_hi = cpb.max(axis=(0, 2)).astype(np.int64)  # [BATCHES]
    sumd = int(d_hi.sum())

    idx_tiles = []
    for c in range(NCORES):
        idx = np.full((128, sumd), ZROW, dtype=np.int32)
        off = 0
        for b in range(BATCHES):
            rows = c * SHARD + b * 128 + np.arange(128)
            d = d_hi[b]
            # fill idx[p, off+s] = s-th in-edge source of node rows[p]
            cnt = counts[rows]
            starts = indptr[rows]
            s_grid = np.arange(d)[None, :]
            take = s_grid < cnt[:, None]
            gather_pos = starts[:, None] + np.minimum(s_grid, np.maximum(cnt[:, None] - 1, 0))
            gather_pos = np.minimum(gather_pos, max(len(psrc_s) - 1, 0))
            vals = psrc_s[gather_pos] if len(psrc_s) else np.zeros_like(gather_pos)
            idx[:, off : off + d] = np.where(take, vals, ZROW)
            off += d
        idx_tiles.append(idx)

    return xs_tab, z1_tabs, dis_tiles, idx_tiles, d_hi, order_per_core


def _build(d_hi):
    from concourse import bacc, bass, mybir, tile
    from concourse.masks import make_identity

    f32 = mybir.dt.float32
    i32 = mybir.dt.int32
    sumd = int(d_hi.sum())

    nc = bacc.Bacc("TRN2", target_bir_lowering=False, debug=False, num_devices=NCORES)

    xs = nc.dram_tensor("xs", [TABROWS, F], f32, kind="ExternalInput")
    z1 = nc.dram_tensor("z1", [SHARD, F], f32, kind="ExternalInput")
    idx = nc.dram_tensor("idx", [128, sumd], i32, kind="ExternalInput")
    dis = nc.dram_tensor("dis", [128, BATCHES], f32, kind="ExternalInput")
    w1 = nc.dram_tensor("w1", [F, F], f32, kind="ExternalInput")
    b1 = nc.dram_tensor("b1", [1, F], f32, kind="ExternalInput")
    w2 = nc.dram_tensor("w2", [F, F], f32, kind="ExternalInput")
    b2 = nc.dram_tensor("b2", [1, F], f32, kind="ExternalInput")
    h2 = nc.dram_tensor("h2", [SHARD, F], f32, kind="ExternalOutput")

    y1_local = nc.dram_tensor("y1_local", [SHARD, F], f32)
    y1_full = nc.dram_tensor("y1_full", [TABROWS, F], f32, addr_space="Shared")

    with tile.TileContext(nc) as tc:
        with (
            tc.tile_pool(name="const", bufs=1) as cpool,
            tc.tile_pool(name="y1pool", bufs=BATCHES) as ypool,
            tc.tile_pool(name="work", bufs=3) as pool,
            tc.tile_pool(name="slots", bufs=2) as spool,
            tc.tile_pool(name="psum", bufs=2, space="PSUM") as psum,
        ):
            ident = cpool.tile([128, 128], f32)
            make_identity(nc, ident[:])
            ones = cpool.tile([1, 128], f32)
            nc.gpsimd.memset(ones[:], 1.0)

            idx_s = cpool.tile([128, sumd], i32)
            nc.sync.dma_start(out=idx_s[:], in_=idx[:])
            dis_s = cpool.tile([128, BATCHES], f32)
            nc.sync.dma_start(out=dis_s[:], in_=dis[:])
            w1_s = cpool.tile([F, F], f32)
            nc.sync.dma_start(out=w1_s[:], in_=w1[:])
            b1_s = cpool.tile([1, F], f32)
            nc.sync.dma_start(out=b1_s[:], in_=b1[:])
            w2_s = cpool.tile([F, F], f32)
            nc.sync.dma_start(out=w2_s[:], in_=w2[:])
            b2_s = cpool.tile([1, F], f32)
            nc.sync.dma_start(out=b2_s[:], in_=b2[:])

            offs = np.concatenate([[0], np.cumsum(d_hi)]).astype(int)
            y1_tiles = []

            def layer(table_ap, wt, bt, self_src, out_sink, first):
                for b in range(BATCHES):
                    d = int(d_hi[b])
                    # alternate columns across two tiles so consecutive
                    # gather writes don't chain on per-tile deps
                    dA = (d + 1) // 2
                    dB = d // 2
                    slotsA = spool.tile([128, dA, F], f32, tag="slotsA")
                    slotsB = spool.tile([128, max(dB, 1), F], f32, tag="slotsB")
                    for s in range(d):
                        col = int(offs[b]) + s
                        tgt = slotsA if s % 2 == 0 else slotsB
                        nc.gpsimd.indirect_dma_start(
                            out=tgt[:, s // 2, :],
                            out_offset=None,
                            in_=table_ap,
                            in_offset=bass.IndirectOffsetOnAxis(
                                ap=idx_s[:, col : col + 1], axis=0
                            ),
                        )
                    segA = pool.tile([128, F], f32, tag="segA")
                    nc.vector.tensor_reduce(
                        out=segA[:],
                        in_=slotsA[:, :dA, :].rearrange("p d f -> p f d"),
                        axis=mybir.AxisListType.X,
                        op=mybir.AluOpType.add,
                    )
                    segB = pool.tile([128, F], f32, tag="segB")
                    nc.vector.tensor_reduce(
                        out=segB[:],
                        in_=slotsB[:, :dB, :].rearrange("p d f -> p f d"),
                        axis=mybir.AxisListType.X,
                        op=mybir.AluOpType.add,
                    )
                    seg = pool.tile([128, F], f32, tag="seg")
                    nc.vector.tensor_tensor(
                        out=seg[:],
                        in0=segA[:],
                        in1=segB[:],
                        op=mybir.AluOpType.add,
                    )
                    agg = pool.tile([128, F], f32, tag="agg")
                    if first:
                        zt = pool.tile([128, F], f32, tag="zt")
                        nc.sync.dma_start(
                            out=zt[:], in_=z1[b * 128 : (b + 1) * 128, :]
                        )
                        nc.vector.scalar_tensor_tensor(
                            out=agg[:],
                            in0=seg[:],
                            scalar=dis_s[:, b : b + 1],
                            in1=zt[:],
                            op0=mybir.AluOpType.mult,
                            op1=mybir.AluOpType.add,
                        )
                    else:
                        t = pool.tile([128, F], f32, tag="t2")
                        nc.vector.tensor_tensor(
                            out=t[:],
                            in0=seg[:],
                            in1=self_src[b][:],
                            op=mybir.AluOpType.add,
                        )
                        nc.vector.tensor_scalar_mul(
                            out=agg[:], in0=t[:], scalar1=dis_s[:, b : b + 1]
                        )
                    psumT = psum.tile([128, 128], f32, tag="pt")
                    nc.tensor.transpose(out=psumT[:], in_=agg[:], identity=ident[:])
                    aggT = pool.tile([128, 128], f32, tag="aggT")
                    nc.scalar.activation(
                        out=aggT[:],
                        in_=psumT[:],
                        func=mybir.ActivationFunctionType.Copy,
                    )
                    ph = psum.tile([128, F], f32, tag="ph")
                    nc.tensor.matmul(
                        ph[:], lhsT=ones[:], rhs=bt[:], start=True, stop=False
                    )
                    nc.tensor.matmul(
                        ph[:], lhsT=aggT[:], rhs=wt[:], start=False, stop=True
                    )
                    if first:
                        y1t = ypool.tile([128, F], f32, tag="y1")
                        nc.scalar.activation(
                            out=y1t[:],
                            in_=ph[:],
                            func=mybir.ActivationFunctionType.Relu,
                            scale=dis_s[:, b : b + 1],
                        )
                        y1_tiles.append(y1t)
                        nc.sync.dma_start(
                            out=y1_local[b * 128 : (b + 1) * 128, :], in_=y1t[:]
                        )
                    else:
                        h2t = pool.tile([128, F], f32, tag="h2t")
                        nc.scalar.activation(
                            out=h2t[:],
                            in_=ph[:],
                            func=mybir.ActivationFunctionType.Relu,
                        )
                        nc.sync.dma_start(
                            out=out_sink[b * 128 : (b + 1) * 128, :], in_=h2t[:]
                        )

            layer(xs[:], w1_s, b1_s, None, None, first=True)

            nc.gpsimd.collective_compute(
                "AllGather",
                bass.mybir.AluOpType.bypass,
                replica_groups=[list(range(NCORES))],
                ins=[y1_local[:]],
                outs=[y1_full[:]],
            )

            layer(y1_full[:], w2_s, b2_s, y1_tiles, h2, first=False)

    nc.compile()
    return nc


def _ensure_ntff_hook():
    """Register the axon NTFF profiling hook if the environment's antenv
    stub lacks the axon_hooks module (otherwise trace=True raises)."""
    import sys as _sys
    import types as _types

    try:
        from antenv.axon_hooks import (  # noqa: F401
            get_axon_ntff_profile_hook,
            set_axon_ntff_profile_hook,
        )
    except ImportError:
        mod = _types.ModuleType("antenv.axon_hooks")
        mod._hook = None

        def set_axon_ntff_profile_hook(h, _mod=mod):
            _mod._hook = h

        def get_axon_ntff_profile_hook(_mod=mod):
            return _mod._hook

        mod.set_axon_ntff_profile_hook = set_axon_ntff_profile_hook
        mod.get_axon_ntff_profile_hook = get_axon_ntff_profile_hook
        _sys.modules["antenv.axon_hooks"] = mod
        try:
            import antenv

            antenv.axon_hooks = mod
        except ImportError:
            pass
    else:
        if get_axon_ntff_profile_hook() is not None:
            return
        from antenv.axon_hooks import set_axon_ntff_profile_hook
    try:
        from trn_agent_boot.trn_boot import _ntff_profile_via_ctypes

        hook = _ntff_profile_via_ctypes("/opt/axon/libaxon_pjrt.so")
        if hook is not None:
            set_axon_ntff_profile_hook(hook)
    except Exception:
        pass  # tracing degrades; compile + run still work


def kernel(x, edges, W1, b1, W2, b2):
    global LAST_RESULTS
    import os

    from concourse.bass_utils import run_bass_kernel_spmd

    _ensure_ntff_hook()

    x = np.asarray(x, dtype=np.float32)
    edges = np.asarray(edges)
    xs_tab, z1_tabs, dis_tiles, idx_tiles, d_hi, order_per_core = _preprocess(x, edges)

    nc = _build(d_hi)

    w1 = np.asarray(W1, dtype=np.float32)
    w2 = np.asarray(W2, dtype=np.float32)
    b1v = np.asarray(b1, dtype=np.float32).reshape(1, F)
    b2v = np.asarray(b2, dtype=np.float32).reshape(1, F)

    in_maps = []
    for c in range(NCORES):
        in_maps.append(
            {
                "xs": xs_tab,
                "z1": z1_tabs[c],
                "idx": idx_tiles[c],
                "dis": dis_tiles[c],
                "w1": w1,
                "b1": b1v,
                "w2": w2,
                "b2": b2v,
            }
        )

    trace = os.environ.get("BASS_TRACE", "1") == "1"
    res = run_bass_kernel_spmd(nc, in_maps, list(range(NCORES)), trace=trace)
    LAST_RESULTS = res

    h2_full = np.empty((N, F), dtype=np.float32)
    for c in range(NCORES):
        h2c = res.results[c]["h2"][:NSHARD]
        h2_full[order_per_core[c]] = h2c
    return (0.5 * (x + h2_full)).astype(np.float32)
